# revision 1
# baseline (speedup 1.0000x reference)
"""Trainium2 Bass kernel for AdvancedEventDKG (TransformerConv + 2xGAT + LSTMCell).

Strategy (8 NeuronCores):
 - Nodes row-sharded: core c owns dst nodes [c*2500, (c+1)*2500).
 - Edges partitioned by destination core; within a core, grouped into
   dst-blocks of 128 nodes, padded to 128-edge tiles.
 - Segment softmax / scatter-add are done with one-hot "S" matmuls on the
   TensorEngine (host uploads the one-hot tiles; pure index data).
 - Per layer, each core all-gathers the (node-feature | a_src) table, then
   dma_gathers the per-edge source rows.
 - Small weights replicated; event-MLP computed on-device (tiny).
"""
import numpy as np
import ml_dtypes

N = 20000
D = 128
H = 128
EV = 64
NH = 4
E = 250000
NT = 8
KACT = 4
CORES = 8
NPC = N // CORES           # 2500 nodes per core
BLK = 128
NBLK = (NPC + BLK - 1) // BLK   # 20 blocks (last partial: 68 rows)
NPCP = NBLK * BLK          # padded local nodes (2560)
RSQRT_H = 1.0 / np.sqrt(128.0)
EPS_SEG = 1e-16

BF = ml_dtypes.bfloat16


def _bf(x):
    return np.ascontiguousarray(np.asarray(x, dtype=np.float32).astype(BF))


def _f32(x):
    return np.ascontiguousarray(np.asarray(x, dtype=np.float32))


def _prep(inputs):
    """Host-side sharding + index prep. Returns (meta, in_maps)."""
    node_ids = np.asarray(inputs["node_ids"])
    emb = np.asarray(inputs["node_emb_table"], dtype=np.float32)[node_ids]  # [N,128]
    src = np.asarray(inputs["src"]).astype(np.int64)
    dst = np.asarray(inputs["dst"]).astype(np.int64)

    # --- per-core edge lists (incl. self-loops), block assignment ---
    order = np.argsort(dst, kind="stable")
    src_s, dst_s = src[order], dst[order]
    core_edges = []  # (esrc, eslot(blocklocal), eblock, is_self)
    for c in range(CORES):
        lo, hi = c * NPC, (c + 1) * NPC
        l = np.searchsorted(dst_s, lo)
        r = np.searchsorted(dst_s, hi)
        es = np.concatenate([src_s[l:r], np.arange(lo, hi)])
        ed = np.concatenate([dst_s[l:r], np.arange(lo, hi)])
        selfm = np.zeros(len(es), dtype=bool)
        selfm[r - l:] = True
        dl = ed - lo
        eb = dl // BLK
        o2 = np.argsort(eb, kind="stable")
        core_edges.append((es[o2], (dl % BLK)[o2], eb[o2], selfm[o2]))

    # per-block tile counts (max over cores -> identical SPMD graph)
    T_pb = np.zeros(NBLK, dtype=np.int64)
    for c in range(CORES):
        eb = core_edges[c][2]
        cnt = np.bincount(eb, minlength=NBLK)
        T_pb = np.maximum(T_pb, -(-cnt // BLK))
    T_pb = T_pb.tolist()
    TE = int(sum(T_pb))  # total tiles per core

    # --- shared (replicated) weight prep ---
    wq = _f32(inputs["wq"]); wk = _f32(inputs["wk"]); wv = _f32(inputs["wv"])
    wskip = _f32(inputs["wskip"])
    wq_t, wq_b = wq[:D], wq[D:]
    wk_t, wk_b = wk[:D], wk[D:]
    wv_t, wv_b = wv[:D], wv[D:]
    wsk_t, wsk_b = wskip[:D], wskip[D:]
    M_all = np.zeros((D, NH * H), np.float32)
    wq_topT = np.zeros((D, NH * H), np.float32)
    wk_topT = np.zeros((D, NH * H), np.float32)
    for h in range(NH):
        s = slice(h * H, (h + 1) * H)
        M_all[:, s] = wq_t[:, s] @ wk_t[:, s].T
        wq_topT[:, s] = wq_t[:, s].T
        wk_topT[:, s] = wk_t[:, s].T
    g0_w = _f32(inputs["g0_w"]); g1_w = _f32(inputs["g1_w"])
    g0_wT = np.zeros((D, NH * H), np.float32)
    g1_wT = np.zeros((D, NH * H), np.float32)
    for h in range(NH):
        s = slice(h * H, (h + 1) * H)
        g0_wT[:, s] = g0_w[:, s].T
        g1_wT[:, s] = g1_w[:, s].T

    onehot = np.zeros((KACT, NT), np.float32)
    onehot[np.arange(KACT), np.asarray(inputs["event_type_ids"])] = 1.0

    shared = {
        "m_all": _bf(M_all),
        "wv_top": _bf(wv_t),
        "wskip_top": _bf(wsk_t),
        "g0_w": _bf(g0_w),
        "g1_w": _bf(g1_w),
        "wih": _bf(inputs["wih"]),
        "lstm_brow": _bf((_f32(inputs["bih"]) + _f32(inputs["bhh"]))[None, :]),
        "brep0": _f32(np.broadcast_to(_f32(inputs["g0_b"])[None, :], (128, 128))),
        "brep1": _f32(np.broadcast_to(_f32(inputs["g1_b"])[None, :], (128, 128))),
        # f32 weights for setup-time math
        "wq_bot": _f32(wq_b), "wk_bot": _f32(wk_b), "wv_bot": _f32(wv_b),
        "wskip_bot": _f32(wsk_b),
        "wq_topT": _f32(wq_topT), "wk_topT": _f32(wk_topT),
        "g0_wT": _f32(g0_wT), "g1_wT": _f32(g1_wT),
        "as0_cols": _f32(_f32(inputs["g0_as"]).T), "ad0_cols": _f32(_f32(inputs["g0_ad"]).T),
        "as1_cols": _f32(_f32(inputs["g1_as"]).T), "ad1_cols": _f32(_f32(inputs["g1_ad"]).T),
        "bq_row": _f32(_f32(inputs["bq"])[None, :]), "bk_row": _f32(_f32(inputs["bk"])[None, :]),
        "bv_row": _f32(_f32(inputs["bv"])[None, :]),
        "bskip_row": _f32(_f32(inputs["bskip"])[None, :]),
        "ef_col": _f32(_f32(inputs["event_features"])[:, None]),
        "ev_w1": _f32(inputs["ev_w1"]), "ev_w2": _f32(inputs["ev_w2"]),
        "b1_row": _f32(_f32(inputs["ev_b1"])[None, :]), "g1_row": _f32(_f32(inputs["ev_g1"])[None, :]),
        "bt1_row": _f32(_f32(inputs["ev_bt1"])[None, :]),
        "b2_row": _f32(_f32(inputs["ev_b2"])[None, :]), "g2_row": _f32(_f32(inputs["ev_g2"])[None, :]),
        "bt2_row": _f32(_f32(inputs["ev_bt2"])[None, :]),
        "intens_col": _f32(np.asarray(inputs["event_intensity"], np.float32)[:, None]),
        "onehot48": _f32(onehot),
        "ett": _f32(inputs["event_type_table"]),
        "idf32": _f32(np.eye(128)),
        "ones_row": _bf(np.ones((1, 128))),
    }

    emb_ext = np.zeros((N, 256), BF)
    emb_ext[:, :D] = emb.astype(BF)
    emb_ext = np.ascontiguousarray(emb_ext)

    in_maps = []
    percore_dbg = []
    for c in range(CORES):
        es, eslot, eb, selfm = core_edges[c]
        # padded per-block edge arrays
        srcp = np.zeros(TE * BLK, np.int64)
        slotp = np.full(TE * BLK, -1, np.int64)
        selfp = np.zeros(TE * BLK, dtype=bool)
        off = 0
        bo = np.bincount(eb, minlength=NBLK)
        pos = np.concatenate([[0], np.cumsum(bo)])
        for b in range(NBLK):
            nbe = int(bo[b])
            srcp[off:off + nbe] = es[pos[b]:pos[b] + nbe]
            slotp[off:off + nbe] = eslot[pos[b]:pos[b] + nbe]
            selfp[off:off + nbe] = selfm[pos[b]:pos[b] + nbe]
            off += T_pb[b] * BLK
        # one-hot S tables, layout [128, TE*128]
        j = np.arange(TE * BLK)
        p = j % BLK
        t = j // BLK
        valid = slotp >= 0
        v1 = valid & (~selfp)
        S1 = np.zeros((BLK, TE * BLK), BF)
        ST1 = np.zeros((BLK, TE * BLK), BF)
        S1[p[v1], t[v1] * BLK + slotp[v1]] = 1
        ST1[slotp[v1], t[v1] * BLK + p[v1]] = 1
        S23 = np.zeros((BLK, TE * BLK), BF)
        ST23 = np.zeros((BLK, TE * BLK), BF)
        S23[p[valid], t[valid] * BLK + slotp[valid]] = 1
        ST23[slotp[valid], t[valid] * BLK + p[valid]] = 1
        # int16 wrapped gather indices
        idxw = np.zeros((128, TE * BLK // 16), np.int16)
        idxw[:16, :] = srcp.astype(np.int16).reshape(-1, 16).T
        idxw[16:, :] = np.tile(idxw[:16, :], (7, 1))
        # local transposed embeddings
        embT = np.zeros((D, NPCP), np.float32)
        embT[:, :NPC] = emb[c * NPC:(c + 1) * NPC].T
        m = {
            "emb_ext": emb_ext,
            "embT_loc": _bf(embT),
            "idx": np.ascontiguousarray(idxw),
            "S1": np.ascontiguousarray(S1), "ST1": np.ascontiguousarray(ST1),
            "S23": np.ascontiguousarray(S23), "ST23": np.ascontiguousarray(ST23),
        }
        m.update(shared)
        in_maps.append(m)
        percore_dbg.append((srcp, slotp, selfp))

    meta = {"T_pb": T_pb, "TE": TE, "emb": emb, "dbg": percore_dbg}
    return meta, in_maps


# ---------------------------------------------------------------------------
# Bass graph builder
# ---------------------------------------------------------------------------

def _build(T_pb):
    import os
    import concourse.bass as bass
    import concourse.mybir as mybir
    from concourse import bacc, tile

    PHASE = int(os.environ.get("KMPHASE", "30"))
    KMBLKS = int(os.environ.get("KMBLKS", str(NBLK)))
    KMLVL = int(os.environ.get("KMLVL", "7"))

    dt = mybir.dt
    AF = mybir.ActivationFunctionType
    ALU = mybir.AluOpType
    TE = int(sum(T_pb))

    nc = bacc.Bacc(None, target_bir_lowering=False, debug=False)

    # ---- DRAM parameters ----
    def inp(name, shape, dty):
        return nc.declare_dram_parameter(name, list(shape), dty, isOutput=False)

    P = {}
    P["emb_ext"] = inp("emb_ext", (N, 256), dt.bfloat16)
    P["embT_loc"] = inp("embT_loc", (D, NPCP), dt.bfloat16)
    P["idx"] = inp("idx", (128, TE * BLK // 16), dt.int16)
    for s in ("S1", "ST1", "S23", "ST23"):
        P[s] = inp(s, (BLK, TE * BLK), dt.bfloat16)
    P["m_all"] = inp("m_all", (D, NH * H), dt.bfloat16)
    P["wv_top"] = inp("wv_top", (D, NH * H), dt.bfloat16)
    P["wskip_top"] = inp("wskip_top", (D, H), dt.bfloat16)
    P["g0_w"] = inp("g0_w", (D, NH * H), dt.bfloat16)
    P["g1_w"] = inp("g1_w", (D, NH * H), dt.bfloat16)
    P["wih"] = inp("wih", (D, 4 * H), dt.bfloat16)
    P["lstm_brow"] = inp("lstm_brow", (1, 4 * H), dt.bfloat16)
    P["brep0"] = inp("brep0", (128, 128), dt.float32)
    P["brep1"] = inp("brep1", (128, 128), dt.float32)
    for s in ("wq_bot", "wk_bot", "wv_bot"):
        P[s] = inp(s, (D, NH * H), dt.float32)
    P["wskip_bot"] = inp("wskip_bot", (D, H), dt.float32)
    for s in ("wq_topT", "wk_topT", "g0_wT", "g1_wT"):
        P[s] = inp(s, (D, NH * H), dt.float32)
    for s in ("as0_cols", "ad0_cols", "as1_cols", "ad1_cols"):
        P[s] = inp(s, (D, NH), dt.float32)
    for s in ("bq_row", "bk_row", "bv_row"):
        P[s] = inp(s, (1, NH * H), dt.float32)
    P["bskip_row"] = inp("bskip_row", (1, H), dt.float32)
    P["ef_col"] = inp("ef_col", (EV, 1), dt.float32)
    P["ev_w1"] = inp("ev_w1", (EV, H), dt.float32)
    P["ev_w2"] = inp("ev_w2", (H, H), dt.float32)
    for s in ("b1_row", "g1_row", "bt1_row", "b2_row", "g2_row", "bt2_row"):
        P[s] = inp(s, (1, H), dt.float32)
    P["intens_col"] = inp("intens_col", (KACT, 1), dt.float32)
    P["onehot48"] = inp("onehot48", (KACT, NT), dt.float32)
    P["ett"] = inp("ett", (NT, H), dt.float32)
    P["idf32"] = inp("idf32", (128, 128), dt.float32)
    P["ones_row"] = inp("ones_row", (1, 128), dt.bfloat16)

    h_out = nc.declare_dram_parameter("h_out", [NPC, H], dt.float32, isOutput=True)
    c_out = nc.declare_dram_parameter("c_out", [NPC, H], dt.float32, isOutput=True)

    f32 = dt.float32
    bf16 = dt.bfloat16

    with tile.TileContext(nc) as tc:
        with (
            tc.tile_pool(name="persist", bufs=1) as pp,
            tc.tile_pool(name="work", bufs=3) as wp,
            tc.tile_pool(name="sml", bufs=4) as sp,
            tc.tile_pool(name="dram", bufs=1, space="DRAM") as dp,
            tc.tile_pool(name="ps_small", bufs=2, space="PSUM") as ps_small,
            tc.tile_pool(name="ps_g", bufs=2, space="PSUM") as ps_g,
            tc.tile_pool(name="ps_scr", bufs=2, space="PSUM") as ps_scr,
        ):
            # ---- load persistent SBUF data ----
            sb = {}
            for name, shape, dty in (
                ("embT_loc", (D, NPCP), bf16),
                ("idx", (128, TE * BLK // 16), dt.int16),
                ("m_all", (D, NH * H), bf16),
                ("wv_top", (D, NH * H), bf16),
                ("wskip_top", (D, H), bf16),
                ("g0_w", (D, NH * H), bf16),
                ("g1_w", (D, NH * H), bf16),
                ("wih", (D, 4 * H), bf16),
                ("lstm_brow", (1, 4 * H), bf16),
                ("brep0", (128, 128), f32),
                ("brep1", (128, 128), f32),
                ("wq_bot", (D, NH * H), f32),
                ("wk_bot", (D, NH * H), f32),
                ("wv_bot", (D, NH * H), f32),
                ("wskip_bot", (D, H), f32),
                ("wq_topT", (D, NH * H), f32),
                ("wk_topT", (D, NH * H), f32),
                ("g0_wT", (D, NH * H), f32),
                ("g1_wT", (D, NH * H), f32),
                ("as0_cols", (D, NH), f32),
                ("ad0_cols", (D, NH), f32),
                ("as1_cols", (D, NH), f32),
                ("ad1_cols", (D, NH), f32),
                ("bq_row", (1, NH * H), f32),
                ("bk_row", (1, NH * H), f32),
                ("bv_row", (1, NH * H), f32),
                ("bskip_row", (1, H), f32),
                ("ef_col", (EV, 1), f32),
                ("ev_w1", (EV, H), f32),
                ("ev_w2", (H, H), f32),
                ("b1_row", (1, H), f32),
                ("g1_row", (1, H), f32),
                ("bt1_row", (1, H), f32),
                ("b2_row", (1, H), f32),
                ("g2_row", (1, H), f32),
                ("bt2_row", (1, H), f32),
                ("intens_col", (KACT, 1), f32),
                ("onehot48", (KACT, NT), f32),
                ("ett", (NT, H), f32),
                ("idf32", (128, 128), f32),
                ("ones_row", (1, 128), bf16),
            ):
                t = pp.tile(list(shape), dty, tag="sb_" + name)
                nc.sync.dma_start(t[:], P[name][:])
                sb[name] = t

            xT_a = pp.tile([D, NPCP], bf16)     # layer-1 output, transposed
            xT_b = pp.tile([D, NPCP], bf16)     # layer-2 output, transposed
            zpad = pp.tile([128, 124], bf16)    # zero pad for xa tables
            nc.vector.memset(zpad[:], 0.0)

            # DRAM intermediates
            xa1_loc = dp.tile([NPC, 256], bf16)
            xa1_full = dp.tile([N, 256], bf16, addr_space="Shared")
            xa2_loc = dp.tile([NPC, 256], bf16)
            xa2_full = dp.tile([N, 256], bf16, addr_space="Shared")

            # =========== event path (setup-time, tiny) ===========
            def t_of(pool, shape, dty, tag=None):
                return pool.tile(list(shape), dty, tag=tag) if tag else pool.tile(list(shape), dty)

            def transpose_small(in_ap, rows, cols, out_dty=f32):
                """in_ap [rows, cols] -> sbuf [cols, rows] (f32 path)."""
                ps = ps_scr.tile([cols, rows], f32, tag="scr")
                idap = sb["idf32"][:rows, :rows]
                nc.tensor.transpose(ps[:], in_ap, idap)
                out = sp.tile([cols, rows], out_dty, tag="evt")
                nc.scalar.activation(out[:], ps[:], AF.Copy)
                return out

            def ln_row(x_sb, g_row, bt_row):
                """LayerNorm along free dim of a [1,128] row, followed by relu."""
                mu = sp.tile([1, 1], f32, tag="evs")
                nc.vector.tensor_reduce(mu[:], x_sb[:], mybir.AxisListType.X, ALU.add)
                nc.vector.tensor_scalar_mul(mu[:], mu[:], 1.0 / H)
                xm = sp.tile([1, H], f32, tag="evt")
                nc.vector.tensor_scalar(xm[:], x_sb[:], mu[:], None, ALU.subtract)
                sq = sp.tile([1, H], f32, tag="evt")
                nc.vector.tensor_mul(sq[:], xm[:], xm[:])
                vs = sp.tile([1, 1], f32, tag="evs")
                nc.vector.tensor_reduce(vs[:], sq[:], mybir.AxisListType.X, ALU.add)
                nc.vector.tensor_scalar(vs[:], vs[:], 1.0 / H, 1e-5, ALU.mult, ALU.add)
                sd = sp.tile([1, 1], f32, tag="evs")
                nc.scalar.activation(sd[:], vs[:], AF.Sqrt)
                rv = sp.tile([1, 1], f32, tag="evs")
                nc.vector.reciprocal(rv[:], sd[:])
                xn = sp.tile([1, H], f32, tag="evt")
                nc.vector.tensor_scalar(xn[:], xm[:], rv[:], None, ALU.mult)
                nc.vector.tensor_mul(xn[:], xn[:], g_row[:])
                nc.vector.tensor_add(xn[:], xn[:], bt_row[:])
                out = sp.tile([1, H], f32, tag="evt")
                nc.scalar.activation(out[:], xn[:], AF.Relu)
                return out

            def mm_row(lhsT, rhs, ncols, brow=None):
                """[1, ncols] = lhsT.T @ rhs (+ brow). Returns sbuf f32 row."""
                ps = ps_scr.tile([1, ncols], f32, tag="scr")
                nc.tensor.matmul(ps[:], lhsT, rhs)
                out = sp.tile([1, ncols], f32, tag="evw")
                if brow is not None:
                    nc.vector.tensor_add(out[:], ps[:], brow)
                else:
                    nc.scalar.activation(out[:], ps[:], AF.Copy)
                return out

            # event MLP
            h1p = mm_row(sb["ef_col"][:], sb["ev_w1"][:], H, sb["b1_row"][:])
            h1 = ln_row(h1p, sb["g1_row"], sb["bt1_row"])
            h1c = transpose_small(h1[:], 1, H)          # [128,1]
            h2p = mm_row(h1c[:], sb["ev_w2"][:], H, sb["b2_row"][:])
            h2 = ln_row(h2p, sb["g2_row"], sb["bt2_row"])
            # intensity-weighted mean of active type embeddings
            w8 = mm_row(sb["intens_col"][:], sb["onehot48"][:], NT)
            nc.vector.tensor_scalar_mul(w8[:], w8[:], 0.25)
            w8c = transpose_small(w8[:], 1, NT)         # [8,1]
            tm = mm_row(w8c[:], sb["ett"][:], H)
            e_row = sp.tile([1, H], f32, tag="evw")
            nc.vector.tensor_add(e_row[:], h2[:], tm[:])
            e_col = transpose_small(e_row[:], 1, H)     # [128,1] f32

            # L1 bias rows: cq/ck/cv rows [1,512]
            cq_row = mm_row(e_col[:], sb["wq_bot"][:], NH * H, sb["bq_row"][:])
            cv_row = mm_row(e_col[:], sb["wv_bot"][:], NH * H, sb["bv_row"][:])
            cskip_row = mm_row(e_col[:], sb["wskip_bot"][:], H, sb["bskip_row"][:])

            # mean over heads of cv + cskip -> skip bias row
            cvm = sp.tile([1, H], f32, tag="evw")
            nc.vector.tensor_add(cvm[:], cv_row[:, 0:H], cv_row[:, H:2 * H])
            nc.vector.tensor_add(cvm[:], cvm[:], cv_row[:, 2 * H:3 * H])
            nc.vector.tensor_add(cvm[:], cvm[:], cv_row[:, 3 * H:4 * H])
            nc.vector.tensor_scalar_mul(cvm[:], cvm[:], 0.25)
            nc.vector.tensor_add(cvm[:], cvm[:], cskip_row[:])
            skiprow_bf = sp.tile([1, H], bf16, tag="evb")
            nc.scalar.activation(skiprow_bf[:], cvm[:], AF.Copy)

            # Note: dst-side additive logit terms (emb_d.u_h + cq_h.ck_h) are
            # constant within each softmax segment and cancel exactly -> only
            # the src-side row term mrow_h = Wk_h @ cq_h is needed.
            cq_cols = [transpose_small(cq_row[:, h * H:(h + 1) * H], 1, H) for h in range(NH)]

            # mrow[1, h*H:...] = (Wk_h @ cq_h)^T
            m_ps = ps_scr.tile([D, NH], f32, tag="scr")
            for h in range(NH):
                nc.tensor.matmul(m_ps[:, h:h + 1], sb["wk_topT"][:, h * H:(h + 1) * H], cq_cols[h][:])
            m_cols = sp.tile([D, NH], f32, tag="evb2")
            nc.scalar.activation(m_cols[:], m_ps[:], AF.Copy)
            mr_ps = ps_scr.tile([1, NH * H], f32, tag="scr")
            for h in range(NH):
                nc.tensor.transpose(mr_ps[:, h * H:(h + 1) * H], m_cols[:, h:h + 1], sb["idf32"][:])
            mrow_bf = sp.tile([1, NH * H], bf16, tag="evb2")
            nc.scalar.activation(mrow_bf[:], mr_ps[:], AF.Copy)

            # GS/GD vectors for the two GAT layers: gs_h = g_w_h @ as_h
            def gsgd(wT_name, cols_name):
                ps = ps_scr.tile([D, NH], f32, tag="scr")
                for h in range(NH):
                    nc.tensor.matmul(ps[:, h:h + 1], sb[wT_name][:, h * H:(h + 1) * H],
                                     sb[cols_name][:, h:h + 1])
                out = sp.tile([D, NH], bf16, tag="evb" + wT_name + cols_name)
                nc.scalar.activation(out[:], ps[:], AF.Copy)
                return out

            GS0 = gsgd("g0_wT", "as0_cols")
            GD0 = gsgd("g0_wT", "ad0_cols")
            GS1 = gsgd("g1_wT", "as1_cols")
            GD1 = gsgd("g1_wT", "ad1_cols")

            # tile offsets per block
            tile_off = [0]
            for b in range(NBLK):
                tile_off.append(tile_off[-1] + T_pb[b])

            # =========== Layer 1: TransformerConv ===========
            with tc.tile_pool(name="ps_bc", bufs=2, space="PSUM") as ps_bc:
                for b in range(KMBLKS):
                    T = T_pb[b]
                    to = tile_off[b]
                    bs = b * BLK
                    rows = min(BLK, NPC - bs)
                    embT_blk = sb["embT_loc"][:, bs:bs + BLK]

                    B_ps = ps_scr.tile([128, 512], f32, tag="scr")
                    nc.tensor.matmul(B_ps[:], embT_blk, sb["m_all"][:], start=True, stop=False)
                    nc.tensor.matmul(B_ps[:], sb["ones_row"][:], mrow_bf[:], start=False, stop=True)
                    B_sb = wp.tile([128, 512], bf16, tag="Bsb")
                    nc.scalar.activation(B_sb[:], B_ps[:], AF.Copy)

                    small = ps_small.tile([128, 512], f32, tag="small")
                    skip_ps = ps_scr.tile([128, 128], f32, tag="scr")
                    nc.tensor.matmul(skip_ps[:], embT_blk, sb["wskip_top"][:], start=True, stop=False)
                    nc.tensor.matmul(skip_ps[:], sb["ones_row"][:], skiprow_bf[:], start=False, stop=True)

                    gb = wp.tile([128, T, 256], bf16, tag="gbuf")
                    s1 = wp.tile([128, T * BLK], bf16, tag="S")
                    st1 = wp.tile([128, T * BLK], bf16, tag="ST")
                    if KMLVL >= 2 and os.environ.get("KMGATHER", "1") == "1":
                        idxb = wp.tile([128, T * 8], dt.int16, tag="idxblk")
                        nc.sync.dma_start(idxb[:], P["idx"][:, to * 8:(to + T) * 8])
                        nc.gpsimd.dma_gather(
                            gb[:], P["emb_ext"][:], idxb[:],
                            num_idxs=T * BLK, num_idxs_reg=T * BLK, elem_size=256,
                            single_packet=False)
                    else:
                        nc.vector.memset(gb[:], 0.0)
                    if KMLVL >= 2 and os.environ.get("KMSDMA", "1") == "1":
                        nc.sync.dma_start(s1[:], P["S1"][:, to * BLK:(to + T) * BLK])
                        nc.sync.dma_start(st1[:], P["ST1"][:, to * BLK:(to + T) * BLK])
                    else:
                        nc.vector.memset(s1[:], 0.0)
                        nc.vector.memset(st1[:], 0.0)

                    gT_ps = ps_g.tile([128, 512], f32, tag="gT")
                    for t in range(T):
                        tsl = slice(t * BLK, (t + 1) * BLK)
                        bt_ps = ps_bc.tile([128, 512], f32, tag="bt")
                        nc.tensor.matmul(bt_ps[:], st1[:, tsl], B_sb[:])
                        alpha_bf = wp.tile([128, NH], bf16, tag="al")
                        tmp = wp.tile([128, 4, 128], bf16, tag="tmp")
                        src_rep = gb[:, t:t + 1, 0:128].broadcast_to([128, 4, 128])
                        nc.vector.tensor_tensor(
                            tmp[:], bt_ps[:].rearrange("p (h c) -> p h c", h=4),
                            src_rep, ALU.mult)
                        lg = wp.tile([128, NH], f32, tag="lg")
                        nc.vector.tensor_reduce(lg[:], tmp[:], mybir.AxisListType.X, ALU.add)
                        nc.scalar.activation(alpha_bf[:], lg[:], AF.Exp, scale=RSQRT_H)
                        if KMLVL >= 5:
                            salpha = wp.tile([128, 4, 128], bf16, tag="sa")
                            s_rep = s1[:, tsl].rearrange("p (o f) -> p o f", o=1).broadcast_to([128, 4, 128])
                            a_rep = alpha_bf[:].rearrange("p (h o) -> p h o", o=1).broadcast_to([128, 4, 128])
                            nc.vector.tensor_tensor(salpha[:], s_rep, a_rep, ALU.mult)
                            nc.tensor.matmul(small[:, 0:4], s1[:, tsl], alpha_bf[:],
                                             start=(t == 0), stop=(t == T - 1))
                            nc.tensor.matmul(gT_ps[:], gb[:, t, 0:128],
                                             salpha[:].rearrange("p h f -> p (h f)"),
                                             start=(t == 0), stop=(t == T - 1))
                    if KMLVL < 5:
                        nc.tensor.matmul(small[:, 0:4], s1[:, 0:BLK], alpha_bf[:])
                        nc.tensor.matmul(gT_ps[:], gb[:, 0, 0:128], bt_sb[:])
                    if KMLVL < 6:
                        continue

                    # combine block
                    rden = wp.tile([128, NH], f32, tag="rden")
                    nc.vector.tensor_scalar(rden[:], small[:, 0:4], EPS_SEG, None, ALU.add)
                    nc.vector.reciprocal(rden[:], rden[:])
                    nc.vector.tensor_scalar_mul(rden[:], rden[:], 0.25)
                    gT_sb = wp.tile([128, 512], bf16, tag="gTsb")
                    nc.scalar.activation(gT_sb[:], gT_ps[:], AF.Copy)
                    outh_ps = ps_scr.tile([128, 512], f32, tag="scr")
                    for h in range(NH):
                        hs = slice(h * H, (h + 1) * H)
                        nc.tensor.matmul(outh_ps[:, hs], gT_sb[:, hs], sb["wv_top"][:, hs])
                    acc0 = wp.tile([128, 128], f32, tag="accA")
                    nc.scalar.activation(acc0[:], skip_ps[:], AF.Copy)
                    accs = [acc0]
                    for h in range(NH):
                        nxt = wp.tile([128, 128], f32, tag="accB" if h % 2 == 0 else "accA")
                        nc.vector.scalar_tensor_tensor(
                            nxt[:], outh_ps[:, h * H:(h + 1) * H], rden[:, h:h + 1],
                            accs[-1][:], ALU.mult, ALU.add)
                        accs.append(nxt)
                    x1_sb = accs[-1]

                    # transpose + xa row build
                    if KMLVL < 7:
                        continue
                    xT_ps = ps_scr.tile([128, 128], f32, tag="scr")
                    nc.tensor.transpose(xT_ps[:], x1_sb[:], sb["idf32"][:])
                    nc.scalar.activation(xT_a[:, bs:bs + BLK], xT_ps[:], AF.Copy)
                    nc.tensor.matmul(small[:, 16:20], xT_a[:, bs:bs + BLK], GS0[:])
                    x1_bf = wp.tile([128, 128], bf16, tag="xbf")
                    nc.scalar.activation(x1_bf[:], x1_sb[:], AF.Copy)
                    as_bf = wp.tile([128, NH], bf16, tag="asbf")
                    nc.scalar.activation(as_bf[:], small[:, 16:20], AF.Copy)
                    nc.sync.dma_start(xa1_loc[bs:bs + rows, 0:128], x1_bf[0:rows, :])
                    nc.sync.dma_start(xa1_loc[bs:bs + rows, 128:132], as_bf[0:rows, :])
                    nc.sync.dma_start(xa1_loc[bs:bs + rows, 132:256], zpad[0:rows, :])

            if PHASE >= 2:
                nc.gpsimd.collective_compute(
                    "AllGather", mybir.AluOpType.bypass,
                    replica_groups=[list(range(CORES))],
                    ins=[xa1_loc.opt()], outs=[xa1_full.opt()])

            # =========== GAT layers ===========
            def gat_layer(xa_full, xT_in, gw_name, GD, brep_name, is_last,
                          xa_out, GS_next, xT_out, ps_te):
                for b in range(NBLK):
                    T = T_pb[b]
                    to = tile_off[b]
                    bs = b * BLK
                    rows = min(BLK, NPC - bs)

                    small = ps_small.tile([128, 512], f32, tag="small")
                    nc.tensor.matmul(small[:, 8:12], xT_in[:, bs:bs + BLK], GD[:])
                    ad_bf = wp.tile([128, NH], bf16, tag="tdsb")
                    nc.scalar.activation(ad_bf[:], small[:, 8:12], AF.Copy)

                    gb = wp.tile([128, T, 256], bf16, tag="gbuf")
                    idxb = wp.tile([128, T * 8], dt.int16, tag="idxblk")
                    nc.sync.dma_start(idxb[:], P["idx"][:, to * 8:(to + T) * 8])
                    nc.gpsimd.dma_gather(
                        gb[:], xa_full[:], idxb[:],
                        num_idxs=T * BLK, num_idxs_reg=T * BLK, elem_size=256,
                        single_packet=False)
                    s23 = wp.tile([128, T * BLK], bf16, tag="S")
                    nc.sync.dma_start(s23[:], P["S23"][:, to * BLK:(to + T) * BLK])
                    st23 = wp.tile([128, T * BLK], bf16, tag="ST")
                    nc.sync.dma_start(st23[:], P["ST23"][:, to * BLK:(to + T) * BLK])

                    gT_ps = ps_g.tile([128, 512], f32, tag="gT")
                    for t in range(T):
                        tsl = slice(t * BLK, (t + 1) * BLK)
                        te_ps = ps_te.tile([128, NH], f32, tag="te")
                        nc.tensor.matmul(te_ps[:], st23[:, tsl], ad_bf[:])
                        z = wp.tile([128, NH], f32, tag="z")
                        nc.vector.tensor_tensor(z[:], gb[:, t, 128:132], te_ps[:], ALU.add)
                        zl = wp.tile([128, NH], f32, tag="zl")
                        nc.vector.scalar_tensor_tensor(zl[:], z[:], 0.2, z[:], ALU.mult, ALU.max)
                        alpha_bf = wp.tile([128, NH], bf16, tag="al")
                        nc.scalar.activation(alpha_bf[:], zl[:], AF.Exp)
                        salpha = wp.tile([128, 4, 128], bf16, tag="sa")
                        s_rep = s23[:, tsl].rearrange("p (o f) -> p o f", o=1).broadcast_to([128, 4, 128])
                        a_rep = alpha_bf[:].rearrange("p (h o) -> p h o", o=1).broadcast_to([128, 4, 128])
                        nc.vector.tensor_tensor(salpha[:], s_rep, a_rep, ALU.mult)
                        nc.tensor.matmul(small[:, 0:4], s23[:, tsl], alpha_bf[:],
                                         start=(t == 0), stop=(t == T - 1))
                        nc.tensor.matmul(gT_ps[:], gb[:, t, 0:128],
                                         salpha[:].rearrange("p h f -> p (h f)"),
                                         start=(t == 0), stop=(t == T - 1))

                    rden = wp.tile([128, NH], f32, tag="rden")
                    nc.vector.tensor_scalar(rden[:], small[:, 0:4], EPS_SEG, None, ALU.add)
                    nc.vector.reciprocal(rden[:], rden[:])
                    nc.vector.tensor_scalar_mul(rden[:], rden[:], 0.25)
                    gT_sb = wp.tile([128, 512], bf16, tag="gTsb")
                    nc.scalar.activation(gT_sb[:], gT_ps[:], AF.Copy)
                    outh_ps = ps_scr.tile([128, 512], f32, tag="scr")
                    for h in range(NH):
                        hs = slice(h * H, (h + 1) * H)
                        nc.tensor.matmul(outh_ps[:, hs], gT_sb[:, hs], sb[gw_name][:, hs])
                    acc0 = wp.tile([128, 128], f32, tag="accA")
                    nc.scalar.activation(acc0[:], outh_ps[:, 0:H], AF.Copy, scale=rden[:, 0:1])
                    accs = [acc0]
                    for h in range(1, NH):
                        nxt = wp.tile([128, 128], f32, tag="accB" if h % 2 == 1 else "accA")
                        nc.vector.scalar_tensor_tensor(
                            nxt[:], outh_ps[:, h * H:(h + 1) * H], rden[:, h:h + 1],
                            accs[-1][:], ALU.mult, ALU.add)
                        accs.append(nxt)
                    xpre = wp.tile([128, 128], f32, tag="accB")
                    nc.vector.tensor_add(xpre[:], accs[-1][:], sb[brep_name][:])
                    x_sb = wp.tile([128, 128], f32, tag="xout")
                    nc.scalar.activation(x_sb[:], xpre[:], AF.Relu)

                    if not is_last:
                        xT_ps = ps_scr.tile([128, 128], f32, tag="scr")
                        nc.tensor.transpose(xT_ps[:], x_sb[:], sb["idf32"][:])
                        nc.scalar.activation(xT_out[:, bs:bs + BLK], xT_ps[:], AF.Copy)
                        nc.tensor.matmul(small[:, 16:20], xT_out[:, bs:bs + BLK], GS_next[:])
                        x_bf = wp.tile([128, 128], bf16, tag="xbf")
                        nc.scalar.activation(x_bf[:], x_sb[:], AF.Copy)
                        as_bf = wp.tile([128, NH], bf16, tag="asbf")
                        nc.scalar.activation(as_bf[:], small[:, 16:20], AF.Copy)
                        nc.sync.dma_start(xa_out[bs:bs + rows, 0:128], x_bf[0:rows, :])
                        nc.sync.dma_start(xa_out[bs:bs + rows, 128:132], as_bf[0:rows, :])
                        nc.sync.dma_start(xa_out[bs:bs + rows, 132:256], zpad[0:rows, :])
                    else:
                        # LSTM cell (h0 = c0 = 0)
                        xT_ps = ps_scr.tile([128, 128], f32, tag="scr")
                        nc.tensor.transpose(xT_ps[:], x_sb[:], sb["idf32"][:])
                        xT3 = wp.tile([128, 128], bf16, tag="xT3")
                        nc.scalar.activation(xT3[:], xT_ps[:], AF.Copy)
                        g_ps = ps_scr.tile([128, 512], f32, tag="scr")
                        nc.tensor.matmul(g_ps[:], xT3[:], sb["wih"][:], start=True, stop=False)
                        nc.tensor.matmul(g_ps[:], sb["ones_row"][:], sb["lstm_brow"][:],
                                         start=False, stop=True)
                        ti = wp.tile([128, 128], f32, tag="ti")
                        nc.scalar.activation(ti[:], g_ps[:, 0:128], AF.Tanh, scale=0.5)
                        si = wp.tile([128, 128], f32, tag="si")
                        nc.vector.tensor_scalar(si[:], ti[:], 0.5, 0.5, ALU.mult, ALU.add)
                        tg = wp.tile([128, 128], f32, tag="tg")
                        nc.scalar.activation(tg[:], g_ps[:, 256:384], AF.Tanh)
                        to_ = wp.tile([128, 128], f32, tag="to")
                        nc.scalar.activation(to_[:], g_ps[:, 384:512], AF.Tanh, scale=0.5)
                        so = wp.tile([128, 128], f32, tag="so")
                        nc.vector.tensor_scalar(so[:], to_[:], 0.5, 0.5, ALU.mult, ALU.add)
                        c_sb = wp.tile([128, 128], f32, tag="c")
                        nc.vector.tensor_mul(c_sb[:], si[:], tg[:])
                        tcn = wp.tile([128, 128], f32, tag="tc")
                        nc.scalar.activation(tcn[:], c_sb[:], AF.Tanh)
                        hh = wp.tile([128, 128], f32, tag="h")
                        nc.vector.tensor_mul(hh[:], so[:], tcn[:])
                        nc.sync.dma_start(h_out[bs:bs + rows, :], hh[0:rows, :])
                        nc.sync.dma_start(c_out[bs:bs + rows, :], c_sb[0:rows, :])

            if PHASE >= 2:
                with tc.tile_pool(name="ps_te", bufs=2, space="PSUM") as ps_te:
                    gat_layer(xa1_full, xT_a, "g0_w", GD0, "brep0", False,
                              xa2_loc, GS1, xT_b, ps_te)
                    if PHASE >= 25:
                        nc.gpsimd.collective_compute(
                            "AllGather", mybir.AluOpType.bypass,
                            replica_groups=[list(range(CORES))],
                            ins=[xa2_loc.opt()], outs=[xa2_full.opt()])
                    if PHASE >= 30:
                        gat_layer(xa2_full, xT_b, "g1_w", GD1, "brep1", True,
                                  None, None, None, ps_te)

    nc.compile()
    return nc


_CACHE = {}


def _run(inputs, trace=False):
    from concourse.bass_utils import run_bass_kernel_spmd
    meta, in_maps = _prep(inputs)
    key = tuple(meta["T_pb"])
    if key not in _CACHE:
        _CACHE[key] = _build(meta["T_pb"])
    nc = _CACHE[key]
    res = run_bass_kernel_spmd(nc, in_maps, core_ids=list(range(CORES)), trace=trace)
    h = np.concatenate([res.results[c]["h_out"] for c in range(CORES)], axis=0)
    c = np.concatenate([res.results[c]["c_out"] for c in range(CORES)], axis=0)
    return h.astype(np.float32), c.astype(np.float32), res


def kernel(**inputs):
    h, c, _ = _run(inputs, trace=False)
    return h, c

